# revision 14
# baseline (speedup 1.0000x reference)
"""Distributed Bass kernel v2: LN + multi-head ALiBi attention + out-proj.

Sharding: 8 cores = (batch b in 0..3) x (head parity g in 0..1).
Core (b, g) computes heads {2i+g : i=0..7} for ALL 2048 queries of batch
b, producing a PARTIAL output (its heads' slice of Wo rows); the host
adds the two partials per batch.  No collectives, no token roll.

One SPMD graph serves all cores: slot i holds head 2i+g; the banded
attention schedule for slot i uses T_sched[i] = band(head 2i+1) >=
band(head 2i), so extents are core-independent; the per-core ALiBi
factor table zeroes whatever the wider schedule over-computes.

Score blocks are WINDOWED: for query chunk qc (512 cols) and key tile
jt (128 rows), only the query window [lo,hi) where |q-j| <= T is
computed, packed side by side into a [128,1024] PSUM arena (bank-
aligned).  exp(S) runs per contiguous span on ACT straight out of
PSUM; the ALiBi bias is applied MULTIPLICATIVELY afterwards:
  softmax numerator = exp(q.k/8) * em,  em[pj,c] = exp(-s_h*|r|)
(1/8 folded into Wq; em is a per-head bf16 table holding the band
profile per diagonal).  PV accumulates the windows into a [65,512]
PSUM tile using pending-zero semantics: each block's PV is split at
the previous block's high-water column so every instruction either
purely accumulates or purely overwrites pending bytes.

The softmax denominator rides as a ones-column of the PV weights; the
8 per-slot [1,512] sums are DMA-gathered into one SBUF tile, one Ln +
per-tile one-hot broadcast matmuls + exp(-x) build 1/l, and O^T is
normalized in place.  Out-proj consumes O^T per 512-query chunk and is
interleaved between attention slots as Tensor-engine filler so the PE
p-state ramp stays hot.
"""

import os
import sys

sys.path.insert(0, "/opt/trn_rl_repo")

import numpy as np
import ml_dtypes

import concourse.bass as bass
import concourse.mybir as mybir
import concourse.tile as tile
from concourse import bacc
from concourse.bass import ts
from concourse.bass_utils import run_bass_kernel_spmd

BF16 = mybir.dt.bfloat16
F32 = mybir.dt.float32
F32R = mybir.dt.float32r

CTX = 2048
DIM = 1024
NH = 16
DH = 64
EPS = 1e-5
NSLOT = 8

LAST_EXEC_NS = None


# ---------------------------------------------------------------------------
# schedule
# ---------------------------------------------------------------------------

def _build_sched(T_sched):
    """Per (slot, qc) block/batch schedule, shared by graph and host data.

    Returns sched[i][qc] = list of batches; each batch is a dict:
      blocks: [(jt, lo, hi, p, segs)]  p = packed col offset in arena
              segs = [(s_lo, s_hi, start, stop)] PV column segments
      spans:  [(p0, p1)] contiguous packed col runs (exp extents)
    em col offset a for a block = EMOFF[i] + lo + (g0 - 128*jt) + T + 127.
    """
    sched = []
    for i in range(NSLOT):
        T = T_sched[i]
        per_qc = []
        for qc in range(4):
            g0 = 512 * qc
            blocks = []
            for jt in range(CTX // 128):
                lo = max(0, 128 * jt - T - g0)
                hi = min(512, 128 * jt + 128 + T - g0)
                if lo < hi:
                    blocks.append((jt, lo, hi))
            # sanity: windows cover [0,512) with nondecreasing lo/hi
            assert blocks[0][1] == 0 and blocks[-1][2] == 512
            # pack into arenas of 1024 cols, blocks bank-aligned (512)
            batches = []
            cur = {"blocks": [], "spans": []}
            p = 0
            span_start = 0
            prev_hi = 0
            for bi, (jt, lo, hi) in enumerate(blocks):
                w = hi - lo
                assert w <= 512
                np_ = p
                if (np_ % 512) + w > 512:  # bank align
                    np_ = (np_ // 512 + 1) * 512
                if np_ + w > 1024:  # arena full
                    cur["spans"].append((span_start, p))
                    batches.append(cur)
                    cur = {"blocks": [], "spans": []}
                    np_ = 0
                    span_start = 0
                elif np_ != p and cur["blocks"]:
                    cur["spans"].append((span_start, p))
                    span_start = np_
                elif not cur["blocks"]:
                    span_start = np_
                p = np_
                last = bi == len(blocks) - 1
                cur["blocks"].append((jt, lo, hi, p, last))
                p += w
                prev_hi = max(prev_hi, hi)
            cur["spans"].append((span_start, p))
            batches.append(cur)
            per_qc.append(batches)
        sched.append(per_qc)
    return sched


# ---------------------------------------------------------------------------
# graph
# ---------------------------------------------------------------------------

def _build_graph(T_sched, emoff, emw):
    nc = bacc.Bacc("TRN2", target_bir_lowering=False, debug=False)
    sched = _build_sched(T_sched)

    x_d = nc.dram_tensor("x", [CTX, DIM], BF16, kind="ExternalInput").ap()
    wq_d = nc.dram_tensor("wq", [8, 128, 4, 128], BF16, kind="ExternalInput").ap()
    wk_d = nc.dram_tensor("wk", [8, 128, 4, 128], BF16, kind="ExternalInput").ap()
    wv_d = nc.dram_tensor("wv", [8, 128, 512], BF16, kind="ExternalInput").ap()
    wo_d = nc.dram_tensor("wo", [4, 128, DIM], BF16, kind="ExternalInput").ap()
    em_d = nc.dram_tensor("em", [128, emw], BF16, kind="ExternalInput").ap()
    idn_d = nc.dram_tensor("ident", [128, 128], BF16, kind="ExternalInput").ap()
    oh_d = nc.dram_tensor("ohsel", [8, 512], F32R, kind="ExternalInput").ap()
    out_d = nc.dram_tensor("out", [CTX, DIM], BF16, kind="ExternalOutput").ap()

    AF = mybir.ActivationFunctionType
    ALU = mybir.AluOpType

    with tile.TileContext(nc) as tc:
        with (
            tc.tile_pool(name="persist", bufs=1) as pp,
            tc.tile_pool(name="xio", bufs=2) as xp,
            tc.tile_pool(name="xnp", bufs=6) as xnp,
            tc.tile_pool(name="small", bufs=4) as sp,
            tc.tile_pool(name="lpool", bufs=2) as lp,
            tc.tile_pool(name="etp", bufs=2) as etp,
            tc.tile_pool(name="ptp", bufs=2) as ptp,
            tc.tile_pool(name="epool", bufs=2) as ep,
            tc.tile_pool(name="opool", bufs=2) as op,
            tc.tile_pool(name="ps_proj", bufs=2, space="PSUM") as ps_proj,
            tc.tile_pool(name="ps_s", bufs=2, space="PSUM") as ps_s,
            tc.tile_pool(name="ps_o", bufs=2, space="PSUM") as ps_o,
        ):
            # ---- persistent SBUF ----
            em_sb = pp.tile([128, emw], BF16, tag="em")
            wq_sb = pp.tile([128, 8, 4, 128], BF16, tag="wq")
            wk_sb = pp.tile([128, 8, 4, 128], BF16, tag="wk")
            wv_sb = pp.tile([128, 8, 512], BF16, tag="wv")
            wot = pp.tile([128, 4, DIM], BF16, tag="wot")
            ident = pp.tile([128, 128], BF16, tag="ident")
            ohsel = pp.tile([8, 512], F32R, tag="ohsel")
            xnT = pp.tile([128, 8, CTX], BF16, tag="xnT")
            KT = pp.tile([128, 4, CTX], BF16, tag="KT")
            QT = pp.tile([128, 4, CTX], BF16, tag="QT")
            Vsb = pp.tile([128, 16, 8, 65], BF16, tag="Vsb")
            OT = pp.tile([128, 4, CTX], BF16, tag="OT")
            eps_sb = pp.tile([128, 1], F32, tag="eps")
            zrow = pp.tile([1, 65], BF16, tag="zrow")

            def persist_loads_early():
                # needed by ch0: ident (transposes), wk/wq/wv (projections)
                nc.gpsimd.dma_start(ident[:], idn_d[:])
                nc.gpsimd.dma_start(wk_sb[:], wk_d.rearrange("k p d m -> p k d m"))
                nc.gpsimd.dma_start(wq_sb[:], wq_d.rearrange("k p d m -> p k d m"))
                nc.gpsimd.dma_start(wv_sb[:], wv_d.rearrange("k p d -> p k d"))

            def persist_loads_late():
                nc.gpsimd.dma_start(em_sb[:], em_d[:])
                nc.gpsimd.dma_start(ohsel[:], oh_d[:])
                nc.gpsimd.dma_start(wot[:], wo_d.rearrange("t p e -> p t e"))

            nc.any.memset(eps_sb[:], EPS)
            nc.any.memset(zrow[:], 0.0)
            nc.any.memset(Vsb[:, :, :, 64:65], 1.0)  # ones col of PV weights

            xcs = [None] * 4
            xnts = [None] * 16
            mv16 = pp.tile([128, 16, 2], F32, tag="mv16")
            rs16 = pp.tile([128, 16], F32, tag="rs16")
            nb16 = pp.tile([128, 16], F32, tag="nb16")

            def px_unit(ch, split=False):
                def f():
                    xc = xp.tile([128, 4, DIM], BF16, tag="xt", name=f"xc{ch}")
                    if split:  # per-tile triggers pipeline stats on ch0
                        for i in range(4):
                            nc.sync.dma_start(
                                xc[:, i, :], x_d[ts(4 * ch + i, 128), :]
                            )
                    else:
                        nc.sync.dma_start(
                            xc[:],
                            x_d[ts(ch, 512), :].rearrange(
                                "(c p) d -> p c d", p=128
                            ),
                        )
                    xcs[ch] = xc
                return f

            # ---- phase-A unit generators (LN, transpose, K/Q/V per chunk) --
            def stats_unit(ch):
                def f():
                    for i in range(4):
                        st6 = sp.tile([128, 2, 6], F32, tag="st6")
                        nc.vector.bn_stats(st6[:, 0, :], xcs[ch][:, i, 0:512])
                        nc.vector.bn_stats(st6[:, 1, :], xcs[ch][:, i, 512:1024])
                        nc.vector.bn_aggr(mv16[:, 4 * ch + i, :], st6[:])
                return f

            def rstd_unit(ch):
                # rstd = sqrt(1/(var+eps)): DVE reciprocal + one Sqrt table
                # (avoids the Ln/Exp table ping-pong on the critical path);
                # nb16 = -mu*rstd only needed by the ACT-Identity xn (ch 2/3)
                def f():
                    o = 4 * ch
                    iv4 = sp.tile([128, 4], F32, tag="iv4")
                    nc.vector.tensor_scalar_add(
                        iv4[:], mv16[:, o:o + 4, 1], EPS
                    )
                    rv4 = sp.tile([128, 4], F32, tag="rv4")
                    nc.vector.reciprocal_approx_fast(rv4[:], iv4[:])
                    nc.scalar.activation(rs16[:, o:o + 4], rv4[:], AF.Sqrt)
                    if ch >= 2:
                        for tt in range(o, o + 4):
                            nc.vector.tensor_scalar(
                                nb16[:, tt:tt + 1], mv16[:, tt, 0:1],
                                rs16[:, tt:tt + 1], -1.0, ALU.mult, ALU.mult,
                            )
                return f

            def xn_unit(ch, redma):
                def f():
                    src = xcs[ch]
                    if redma:
                        src = xp.tile([128, 4, DIM], BF16, tag="xt", name=f"xr{ch}")
                        nc.sync.dma_start(
                            src[:],
                            x_d[ts(ch, 512), :].rearrange("(c p) d -> p c d", p=128),
                        )
                    for i in range(4):
                        tt = 4 * ch + i
                        xn = xnp.tile([128, DIM], BF16, tag="xn")
                        if ch < 2:
                            # DVE path: (x - mu) * rstd, off the ACT queue
                            nc.vector.tensor_scalar(
                                xn[:], src[:, i, :], mv16[:, tt, 0:1],
                                rs16[:, tt:tt + 1], ALU.subtract, ALU.mult,
                            )
                        else:
                            # ACT path (DVE is busy with attention here)
                            nc.scalar.activation(
                                xn[:], src[:, i, :], AF.Identity,
                                bias=nb16[:, tt:tt + 1], scale=rs16[:, tt:tt + 1],
                            )
                        xnts[tt] = xn
                return f

            def tr_unit(ch, half):
                # per token-tile: 4 ko transposes -> one strided copy
                def f():
                    for i in range(4):
                        tt = 4 * ch + i
                        tp = ps_proj.tile([128, 512], F32, tag="pp")
                        tpv = tp[:].bitcast(BF16)
                        for kk, ko in enumerate(range(4 * half, 4 * half + 4)):
                            nc.tensor.transpose(
                                tpv[:, ts(kk, 128)],
                                xnts[tt][:, ts(ko, 128)], ident[:],
                            )
                        nc.vector.tensor_copy(
                            xnT[:, 4 * half:4 * half + 4, ts(tt, 128)],
                            tpv[:, 0:512].rearrange("p (k t) -> p k t", t=128),
                        )
                return f

            def kq_unit(ch, which, half):
                w_sb, dst = (wk_sb, KT) if which == "k" else (wq_sb, QT)

                def f():
                    for dqt in range(2 * half, 2 * half + 2):
                        ps = ps_proj.tile([128, 512], F32, tag="pp")
                        for ko in range(8):
                            nc.tensor.matmul(
                                ps[:], w_sb[:, ko, dqt, :], xnT[:, ko, ts(ch, 512)],
                                start=(ko == 0), stop=(ko == 7),
                            )
                        eng = nc.vector if (dqt % 2 == 0) else nc.scalar
                        if eng is nc.scalar:
                            nc.scalar.activation(dst[:, dqt, ts(ch, 512)], ps[:], AF.Copy)
                        else:
                            nc.vector.tensor_copy(dst[:, dqt, ts(ch, 512)], ps[:])
                return f

            def v_unit(ch, half):
                def f():
                    for jt in range(4 * ch + 2 * half, 4 * ch + 2 * half + 2):
                        ps = ps_proj.tile([128, 512], F32, tag="pp")
                        for ko in range(8):
                            nc.tensor.matmul(
                                ps[:], xnT[:, ko, ts(jt, 128)], wv_sb[:, ko, :],
                                start=(ko == 0), stop=(ko == 7),
                            )
                        nc.vector.tensor_copy(
                            Vsb[:, jt, :, 0:64],
                            ps[:].rearrange("p (i d) -> p i d", d=64),
                        )
                return f

            def ch_units(ch, redma=False):
                return [
                    xn_unit(ch, redma), tr_unit(ch, 0), tr_unit(ch, 1),
                    kq_unit(ch, "k", 0), kq_unit(ch, "k", 1),
                    kq_unit(ch, "q", 0), kq_unit(ch, "q", 1),
                    v_unit(ch, 0), v_unit(ch, 1),
                ]

            # ---- attention ----
            lcols = {}

            def attn_units(i, qc):
                """Yield (A, B) unit pairs per arena batch + a drain unit."""
                T = T_sched[i]
                t, base = i // 2, 64 * (i % 2)
                g0 = 512 * qc
                batches = sched[i][qc]
                po = {}

                def mk_scores(bt, arena):
                    def f():
                        for (jt, lo, hi, p, _l) in bt["blocks"]:
                            w = hi - lo
                            nc.tensor.matmul(
                                arena[:, p:p + w],
                                KT[base:base + 64, t, ts(jt, 128)],
                                QT[base:base + 64, t, g0 + lo:g0 + hi],
                                start=True, stop=True,
                            )
                        et = etp.tile([128, 1024], BF16, tag="et")
                        for (s0, s1) in bt["spans"]:
                            nc.scalar.activation(et[:, s0:s1], arena[:, s0:s1], AF.Exp)
                        pt = ptp.tile([128, 1024], BF16, tag="pt")
                        eng = nc.gpsimd if i < 3 else nc.vector
                        for (jt, lo, hi, p, _l) in bt["blocks"]:
                            w = hi - lo
                            a = emoff[i] + lo + (g0 - 128 * jt) + T + 127
                            eng.tensor_tensor(
                                pt[:, p:p + w], et[:, p:p + w],
                                em_sb[:, a:a + w], ALU.mult,
                            )
                        po["pt"] = pt
                    return f

                def mk_pv(bt):
                    def f():
                        pt = po["pts"].pop(0)
                        if not po.get("zeroed"):
                            # start=True full-width zero write: one cheap
                            # matmul resets the bank, every PV accumulates
                            nc.tensor.matmul(
                                po["tile"][0:65, :], zrow[:],
                                em_sb[0:1, 0:512],
                                start=True, stop=False,
                            )
                            po["zeroed"] = True
                        for (jt, lo, hi, p, last) in bt["blocks"]:
                            nc.tensor.matmul(
                                po["tile"][0:65, lo:hi],
                                Vsb[:, jt, i, :],
                                pt[:, p:p + hi - lo],
                                start=False, stop=last,
                            )
                    return f

                po["pts"] = []
                units = []
                for bt in batches:
                    arena = None

                    def mk_a(bt=bt):
                        def f():
                            if po.get("tile") is None:
                                po["tile"] = ps_o.tile(
                                    [128, 512], F32, tag="po", name=f"po{i}_{qc}"
                                )
                            ar = ps_s.tile([128, 1024], F32, tag="arena")
                            mk_scores(bt, ar)()
                            po["pts"].append(po.pop("pt"))
                        return f

                    units.append((mk_a(), mk_pv(bt)))

                def drain():
                    pot = po["tile"]
                    if i % 2 == 0:
                        nc.vector.tensor_copy(OT[0:64, t, g0:g0 + 512], pot[0:64, :])
                    else:
                        tmp = ep.tile([64, 512], BF16, tag="otmp", name=f"otmp{i}_{qc}")
                        nc.scalar.copy(tmp[:], pot[0:64, :])
                        nc.sync.dma_start(OT[64:128, t, g0:g0 + 512], tmp[:])
                    lt = lp.tile([1, 512], F32, tag="lt", name=f"lt{i}_{qc}")
                    if i % 2 == 0:
                        nc.vector.tensor_copy(lt[:], pot[64:65, :])
                    else:
                        nc.scalar.copy(lt[:], pot[64:65, :])
                    nc.sync.dma_start(lcols[qc][i:i + 1, :], lt[:])
                return units, drain

            def norm_unit(qc):
                def f():
                    g0 = 512 * qc
                    linv = lp.tile([8, 512], F32, tag="lnl")
                    nc.vector.reciprocal_approx_fast(linv[:], lcols[qc][:])
                    linvr = lp.tile([8, 512], F32R, tag="lnlr")
                    nc.vector.tensor_copy(linvr[:], linv[:])
                    for t in range(4):
                        pb = ps_proj.tile([128, 512], F32, tag="pp")
                        nc.tensor.matmul(
                            pb[:], ohsel[:, ts(t, 128)], linvr[:],
                            start=True, stop=True,
                        )
                        einv = ep.tile([128, 512], BF16, tag="einv")
                        nc.scalar.activation(einv[:], pb[:], AF.Copy)
                        nc.vector.tensor_tensor(
                            OT[:, t, g0:g0 + 512], OT[:, t, g0:g0 + 512],
                            einv[:], ALU.mult,
                        )
                return f

            def outproj_units(qc):
                units = []
                for it in range(4 * qc, 4 * qc + 4):
                    for ec in range(2):
                        def f(it=it, ec=ec):
                            ps = ps_proj.tile([128, 512], F32, tag="pp")
                            for t in range(4):
                                nc.tensor.matmul(
                                    ps[:], OT[:, t, ts(it, 128)],
                                    wot[:, t, ts(ec, 512)],
                                    start=(t == 0), stop=(t == 3),
                                )
                            oc = op.tile([128, 512], BF16, tag="oc")
                            if ec == 0:
                                nc.vector.tensor_copy(oc[:], ps[:])
                            else:
                                nc.scalar.activation(oc[:], ps[:], AF.Copy)
                            nc.sync.dma_start(out_d[ts(it, 128), ts(ec, 512)], oc[:])
                        units.append(f)
                return units

            def weave(slot_list, fillers, skip=0):
                """Emit attention slots with filler units between A and B.

                skip: number of leading A/B pairs that get no filler, so
                fillers whose deps resolve late (norm chains) don't
                head-of-line-block the PE queue.
                """
                fi = [0]
                n = [0]

                def fill():
                    if n[0] >= skip and fi[0] < len(fillers):
                        fillers[fi[0]]()
                        fi[0] += 1
                    n[0] += 1

                for (i, qc) in slot_list:
                    units, drain = attn_units(i, qc)
                    for (A, B) in units:
                        A()
                        fill()
                        B()
                    drain()
                while fi[0] < len(fillers):
                    fillers[fi[0]]()
                    fi[0] += 1

            # ---- emission ----
            for qc in range(4):
                lcols[qc] = lp.tile([8, 512], F32, tag="lcol", name=f"lcol{qc}")

            px_unit(0, split=True)()
            px_unit(1, split=True)()
            persist_loads_early()
            stats_unit(0)()
            rstd_unit(0)()
            persist_loads_late()
            u0 = ch_units(0)
            u0[0]()  # xn(0)
            u0[1](); u0[2]()  # transposes(0)
            px_unit(2)()
            stats_unit(1)()
            rstd_unit(1)()
            for u in u0[3:]:
                u()
            u1 = ch_units(1)
            u1[0](); u1[1](); u1[2]()
            px_unit(3)()
            stats_unit(2)()
            rstd_unit(2)()
            for u in u1[3:]:
                u()
            stats_unit(3)()
            rstd_unit(3)()
            weave([(i, 0) for i in range(7)], ch_units(2, redma=True))
            weave([(i, 1) for i in range(7)], ch_units(3, redma=True))
            weave([(7, 0)], [])
            weave([(7, 1)], [norm_unit(0)] + outproj_units(0))
            weave([(i, 2) for i in range(8)], [norm_unit(1)] + outproj_units(1),
                  skip=2)
            weave([(i, 3) for i in range(8)], [norm_unit(2)] + outproj_units(2),
                  skip=2)
            norm_unit(3)()
            for u in outproj_units(3):
                u()

    nc.compile()
    return nc


# ---------------------------------------------------------------------------
# host prep
# ---------------------------------------------------------------------------

def _prep(x, ln_w, ln_b, Wq, Wk, Wv, Wo, M):
    x = np.asarray(x, np.float32)
    ln_w = np.asarray(ln_w, np.float32)
    ln_b = np.asarray(ln_b, np.float32)
    Wq = np.asarray(Wq, np.float32)
    Wk = np.asarray(Wk, np.float32)
    Wv = np.asarray(Wv, np.float32)
    Wo = np.asarray(Wo, np.float32)
    M = np.asarray(M, np.float32)
    assert not np.any(ln_b), "kernel assumes ln_b == 0"

    s_heads = (-M[:, 0, 1]).astype(np.float64)  # M[h,0,1] = -s_h
    Ts = [min(CTX, int(np.ceil(19.0 / s))) for s in s_heads]
    T_sched = [Ts[2 * i + 1] for i in range(NSLOT)]

    # em table geometry
    emoff = []
    off = 0
    for i in range(NSLOT):
        emoff.append(off)
        off += 2 * T_sched[i] + 384  # W_i = 2T+382, pad 2
    emw = off

    wq_eff = (ln_w[:, None] * Wq) / 8.0
    wk_eff = ln_w[:, None] * Wk
    wv_eff = ln_w[:, None] * Wv

    def cols_for(g):  # column gather for this core's 8 heads, slot-major
        idx = []
        for i in range(NSLOT):
            h = 2 * i + g
            idx.extend(range(DH * h, DH * h + DH))
        return np.asarray(idx)

    def kq_layout(w):  # [1024, 512] -> [ko, p, dqt, m]
        return np.ascontiguousarray(
            w.reshape(8, 128, 4, 128).transpose(0, 1, 2, 3)
        ).astype(ml_dtypes.bfloat16)

    ident = np.eye(128, dtype=np.float32).astype(ml_dtypes.bfloat16)
    ohsel = np.zeros((8, 512), np.float32)
    for t in range(4):
        ohsel[2 * t, 128 * t:128 * t + 64] = 1.0
        ohsel[2 * t + 1, 128 * t + 64:128 * (t + 1)] = 1.0

    pj = np.arange(128, dtype=np.float64)[:, None]

    in_maps = []
    for c in range(8):
        b, g = c // 2, c % 2
        idx = cols_for(g)
        wq_a = kq_layout(wq_eff[:, idx])
        wk_a = kq_layout(wk_eff[:, idx])
        wv_a = np.ascontiguousarray(
            wv_eff[:, idx].reshape(8, 128, 512)
        ).astype(ml_dtypes.bfloat16)
        wo_a = np.ascontiguousarray(
            Wo[idx, :].reshape(4, 128, DIM)
        ).astype(ml_dtypes.bfloat16)

        em = np.zeros((128, emw), np.float64)
        for i in range(NSLOT):
            h = 2 * i + g
            T = T_sched[i]
            W = 2 * T + 382
            y = np.arange(W, dtype=np.float64)[None, :]
            r = y - pj - (T + 127)
            em[:, emoff[i]:emoff[i] + W] = np.exp(-s_heads[h] * np.abs(r))
        em_a = em.astype(np.float32).astype(ml_dtypes.bfloat16)

        xr = np.ascontiguousarray(x[b]).astype(ml_dtypes.bfloat16)
        in_maps.append({
            "x": xr, "wq": wq_a, "wk": wk_a, "wv": wv_a, "wo": wo_a,
            "em": em_a, "ident": ident, "ohsel": ohsel,
        })
    return T_sched, emoff, emw, in_maps


def kernel(**inputs):
    global LAST_EXEC_NS
    T_sched, emoff, emw, in_maps = _prep(**inputs)
    nc = _build_graph(T_sched, emoff, emw)
    trace = os.environ.get("KERNEL_TRACE") == "1"
    res = run_bass_kernel_spmd(
        nc, in_maps, core_ids=list(range(8)), trace=trace
    )
    LAST_EXEC_NS = res.exec_time_ns
    out = np.empty((4, CTX, DIM), np.float32)
    for b in range(4):
        out[b] = (
            np.asarray(res.results[2 * b]["out"], np.float32)
            + np.asarray(res.results[2 * b + 1]["out"], np.float32)
        )
    return out
